# revision 32
# baseline (speedup 1.0000x reference)
"""Trainium2 Bass kernel for BaseBertSelfAttention (B=2, S=2048, H=1024, 16 heads).

Sharding (8 NeuronCores):
  - Tensor-parallel on heads: core c owns heads (2c, 2c+1) -> d_local = 128.
  - Each core: QKV projections (column-parallel) for its 2 heads over BOTH
    batches, attention in transposed layout (scores^T: keys on partitions,
    queries on the free axis), softmax denominator via a ones-augmented V
    column (weight column 0, so the denominator lands on PSUM partition 0
    where reciprocal_approx_fast reads it partition-matched -- the custom
    DVE op misreads non-zero base partitions on HW), normalized context
    ctx^T [d_local=128, B*S].
  - FOUR AllToAlls (one per half-batch, 128KB each) redistribute ctx^T from
    head-sharding to row-sharding.  Each one-row-tile Wo/LayerNorm tail is
    emitted one attention stage AFTER its collective completes: the
    per-engine instruction queues are in-order, so a tail's Wo matmuls
    must only reach the head of the PE queue once its ctxf is already
    resident (hardware-measured: an early tail stalls the whole PE ~10us
    behind the collective).  Only the final collective plus one tail
    (~20-30 us, collective-warmth dependent) is exposed at the end.
  - Core c owns rows [128c, 128c+128) of each half-batch (512 rows total).

Hardware-profiled bottleneck structure (NTFF traces, not the simulator --
the sim is ~25% optimistic and mis-ranks engines): the softmax exp stream
(16.8M elements/core, 128 ACTIVATEs of 1024 elem/partition at
0.833ns/elem + 260ns/call) is ~143us of ACT time; the PE runs ~176us
residency (scores already row-packed pairwise via base_partition 0/64 ->
concurrent tiles).  Everything else is latency engineering: startup races
to the first exp at ~20us (framework preamble is 7us; front loads split
across the sync+scalar DMA queues with partition-major host layouts for
>=1KB DMA packets), shard-boundary normalize chains are kept off the
critical path (reciprocal_approx_fast at 0.67us vs 3.35us for the slow
cross-partition reciprocal; copies drain the ctx PSUM banks before the
eb-broadcast/cn multiply chain), and mid-kernel tails run their ctxf loads
on the scalar queue so the sync queue stays clear for shard cn stores.

Precision: fp8 (e4m3) matmul inputs with DoubleRow packing for the QKV
projections, the probs@V matmul and the Wo projection; bf16 Q^T/K^T for
the scores matmul; fp32 PSUM accumulation and softmax denominators; bf16
LayerNorm epilogue and bf16 output (2x DVE rate; host converts back to
f32).  Scale bookkeeping: x^T is pre-scaled by SX=16, Q/K rescale at the
bias-add, V carries SX with a 1/SX denominator column, Wo carries 256 and
the residual 65536 -- the common row scale cancels in LayerNorm.
NaN traps on HW (sim-clean, hardware-broken): fp32 matmuls (FP32HI
two-pass) interleaved into the fp8-DR PE stream, and in-place or
cross-partition reciprocal_approx_fast.

All inputs pack into ONE fp8 blob (f32 section bitcast on device): each
extra PJRT buffer costs ~30-150us/dispatch through the axon tunnel.
Hardware-verified: device exec ~250-258us (NTFF, core 0; was 300.6us),
relative error 2.6e-3 (gate 2e-2).
"""

import numpy as np
import ml_dtypes

import concourse.bass as bass
import concourse.tile as tile
from concourse import bacc, mybir
from concourse.bass_utils import run_bass_kernel_spmd

BF16 = mybir.dt.bfloat16
FP8 = mybir.dt.float8e4
F32 = mybir.dt.float32
AF = mybir.ActivationFunctionType
DR = mybir.MatmulPerfMode.DoubleRow
P = 128

B, S, H = 2, 2048, 1024
NH, HD = 16, 64
NCORES = 8
EPS = 1e-12
SCALE = 1.0 / 8.0   # 1/sqrt(HD)
SX = 16.0           # fp8 x^T pre-scale (host side)
SVC = 1.0 / SX      # denominator column constant: cn = 256 * v_avg

_CACHE: dict = {}


def _build_program(s=S):
    """Build the (identical-across-cores) Bass program."""
    nkb = s // P               # key blocks of 128 (16)
    qc_per_b = NCORES // B     # q chunks per batch (4)
    qw = (B * s) // NCORES     # q-chunk width (512)
    rpb = s // NCORES          # output rows per core per batch (256)
    ho = H // P                # h chunks of 128 (8)

    nc = bacc.Bacc("TRN2", target_bir_lowering=False, debug=False,
                   num_devices=NCORES)
    # All inputs are packed into ONE flat fp8 blob: each extra PJRT input
    # buffer costs ~30-150 us of per-dispatch overhead through the axon
    # tunnel.  The f32 section is stored as raw bytes and bitcast on
    # device (the fp8 section's total size is 4B-aligned).
    n8 = {"xT": B * H * s, "wqk": H * 2 * P, "wv": H * P, "wo": H * H}
    n32 = {"bqk": P * 2, "bv": P * P, "maskT": B * P * nkb,
           "xres": 2 * rpb * H, "gamma": P * H, "beta": P * H}
    N8 = sum(n8.values())
    assert N8 % 4 == 0
    blob = nc.dram_tensor("blob", [N8 + 4 * sum(n32.values())], FP8,
                          kind="ExternalInput")

    def views(sizes, base, scale, cast):
        o, vs = 0, {}
        for k, n in sizes.items():
            ap = blob[base + scale * o:base + scale * (o + n)]
            vs[k] = ap.bitcast(F32) if cast else ap
            o += n
        return vs

    # host-side layouts are partition-major (SBUF layout) so every DMA
    # reads long contiguous runs per partition instead of 128B packets
    v8 = views(n8, 0, 1, cast=False)
    v32 = views(n32, N8, 4, cast=True)
    xT = v8["xT"].rearrange("(p b o ss) -> p b o ss", p=P, b=B, o=H // P)
    wqk = v8["wqk"].rearrange("(t p o d) -> t p o d", t=2, p=P, o=H // P)
    wv = v8["wv"].rearrange("(p o d) -> p o d", p=P, o=H // P)
    wo = v8["wo"].rearrange("(p o n) -> p o n", p=P, o=H // P)
    bqk = v32["bqk"].rearrange("(p t) -> p t", p=P)
    bv = v32["bv"].rearrange("(p d) -> p d", p=P)
    maskT = v32["maskT"].rearrange("(p b k) -> p b k", p=P, b=B)
    xres = v32["xres"].rearrange("(p r hh) -> p r hh", p=P, hh=H)
    gamma = v32["gamma"].rearrange("(p hh) -> p hh", p=P)
    beta = v32["beta"].rearrange("(p hh) -> p hh", p=P)
    out = nc.dram_tensor("out", [2 * rpb, H], BF16, kind="ExternalOutput")

    with tile.TileContext(nc) as tc:
        _kernel_body(
            tc, s, nkb, qw, qc_per_b, rpb, ho,
            xT, wqk, wv, wo, bqk, bv, maskT, xres, gamma, beta, out,
        )
    nc.compile()
    return nc


def _kernel_body(tc, s, nkb, qw, qc_per_b, rpb, ho,
                 xT, wqk, wv, wo, bqk, bv, maskT, xres, gamma, beta, out):
    nc = tc.nc
    VPAD = 80  # padded free width of the ones-augmented V tiles (65 used)
    nkp = nkb // 2  # key-block pairs (8) for DoubleRow AV

    import contextlib
    stack = contextlib.ExitStack()
    with stack:
        consts = stack.enter_context(tc.tile_pool(name="consts", bufs=1))
        dram = stack.enter_context(tc.tile_pool(name="dram", bufs=1, space="DRAM"))

        # ---------------- constant / input loads ----------------
        # Startup is latency-critical (the softmax ScalarEngine stream is the
        # kernel bottleneck, so its first exp should start ASAP).  Split the
        # front loads across BOTH DMA queues: sync (SP) takes wqk + the even
        # x^T chunks, the scalar-engine queue (idle before the first exp)
        # takes bqk/mask + the odd x^T chunks + V-path constants.
        wqk_sb = consts.tile([P, ho, 2, P], FP8)
        # two DMAs into one tile: the Q half lands first so the very first
        # projection matmul isn't gated on the K half
        nc.sync.dma_start(wqk_sb[:, :, 0, :], wqk[0])
        nc.sync.dma_start(wqk_sb[:, :, 1, :], wqk[1])
        wq_sb = wqk_sb[:, :, 0, :]
        wk_sb = wqk_sb[:, :, 1, :]

        bqk_sb = consts.tile([P, 2], F32)
        nc.scalar.dma_start(bqk_sb, bqk[:, :])
        bq_sb = bqk_sb[:, 0:1]
        bk_sb = bqk_sb[:, 1:2]
        mask_sb = consts.tile([P, B, nkb], F32)
        nc.scalar.dma_start(mask_sb, maskT[:, :, :])

        xT_sb = consts.tile([P, B, ho, s], FP8)
        xT_r = xT
        # x^T (fp8, host-scaled by SX): first s-chunk of batch 0 across all o
        for o in range(0, ho, 2):
            nc.sync.dma_start(xT_sb[:, 0, o, 0:512], xT_r[:, 0, o, 0:512])
            nc.scalar.dma_start(
                xT_sb[:, 0, o + 1, 0:512], xT_r[:, 0, o + 1, 0:512])

        wv_sb = consts.tile([P, ho, P], FP8)
        nc.scalar.dma_start(wv_sb, wv[:, :, :])
        bv_b = consts.tile([P, P], F32)
        nc.scalar.dma_start(bv_b, bv[:, :])

        # rest of batch-0 x^T, then batch 1 (sync queue; scalar queue must
        # stay clear once the softmax exp stream begins)
        for o in range(ho):
            nc.sync.dma_start(xT_sb[:, 0, o, 512:s], xT_r[:, 0, o, 512:s])
        for o in range(ho):
            nc.sync.dma_start(xT_sb[:, 1, o, :], xT_r[:, 1, o, :])

        wo_sb = consts.tile([P, ho, H], FP8)
        ones_sb = consts.tile([1, 65], BF16)
        nc.vector.memset(ones_sb, 1.0)
        eps_sb = consts.tile([P, 1], F32)
        nc.vector.memset(eps_sb, EPS)

        # gamma/beta on the gpsimd DMA queue (idle at startup; only needed
        # by the tails), keeping the sync/scalar queues for the race to the
        # first exp
        gamma_b = consts.tile([P, H], F32)
        nc.sync.dma_start(gamma_b, gamma[:, :])
        beta_b = consts.tile([P, H], F32)
        nc.sync.dma_start(beta_b, beta[:, :])

        xres_sb = consts.tile([P, 2 * rpb // P, H], F32)

        # attention intermediates
        qT_sb = consts.tile([P, B, s], BF16)   # Q^T [d_local, b, s] true scale
        kT_sb = consts.tile([P, B, s], BF16)   # K^T [d_local, b, s] true scale
        # ones-augmented V (natural layout), per head:
        #   [p(s-inner), b, kb-pair, 2, VPAD] fp8, value scale SX
        v_e = consts.tile([P, B, nkp, 2, VPAD], FP8)
        v_o = consts.tile([P, B, nkp, 2, VPAD], FP8)
        # only the denominator column needs initialization (cols 1:65 are
        # overwritten by the V bias-add; cols 65: are never read):
        # cn = ctx*recip lands at 256*v_avg in fp8 range.  The denominator
        # sits in WEIGHT COLUMN 0 so it lands on PSUM PARTITION 0, where
        # reciprocal_approx_fast reads it on the fast partition-matched path
        # (the custom DVE op misreads non-zero base partitions).
        nc.vector.memset(v_e[:, :, :, :, 0:1], SVC)
        nc.vector.memset(v_o[:, :, :, :, 0:1], SVC)

        # A2A bounce buffers (DRAM, local), one pair per HALF-batch: four
        # small collectives let the first three (and their tails) hide under
        # the softmax plateau; only the last 128KB exchange is exposed
        rhb = rpb // 2   # rows per core per half-batch (128)
        a2a_in = [dram.tile([NCORES * P, rhb], FP8, name=f"a2a_in{hb}")
                  for hb in range(2 * B)]
        a2a_out = [dram.tile([NCORES * P, rhb], FP8, name=f"a2a_out{hb}")
                   for hb in range(2 * B)]

        # PSUM: qk pool 1 bank (QKV proj), s pool 2x2 banks (shared with the
        # tails' Wo matmuls), ctx pool 2 banks, eb pool 1 bank = 8 banks.
        # Keeping eb out of ps_ctx lets the next shard's AV accumulation
        # start as soon as the ctx tiles are drained (copy+recip), instead
        # of waiting for the full normalize chain.
        ps_qk = stack.enter_context(tc.tile_pool(name="ps_qk", bufs=1, space="PSUM"))
        ps_s = stack.enter_context(tc.tile_pool(name="ps_s", bufs=2, space="PSUM"))
        ps_ctx = stack.enter_context(tc.tile_pool(name="ps_ctx", bufs=2, space="PSUM"))
        ps_eb = stack.enter_context(tc.tile_pool(name="ps_eb", bufs=1, space="PSUM"))
        ptile = stack.enter_context(tc.tile_pool(name="ptile", bufs=6))
        misc = stack.enter_context(tc.tile_pool(name="misc", bufs=3))
        fin = stack.enter_context(tc.tile_pool(name="fin", bufs=2))

        def qkv_stage(b):
            # Q/K chunk projections first (attention's scores need them
            # immediately); V blocks after (first consumed only after the
            # first softmax exp)
            for sc in range(s // 512):
                sl = slice(sc * 512, (sc + 1) * 512)
                for w_sb, bias_sb, dst in (
                    (wq_sb, bq_sb, qT_sb),
                    (wk_sb, bk_sb, kT_sb),
                ):
                    ps = ps_qk.tile([P, 512], F32, tag="qk")
                    for op in range(ho // 2):
                        nc.tensor.matmul(
                            ps, lhsT=w_sb[:, 2 * op:2 * op + 2, :].opt(),
                            rhs=xT_sb[:, b, 2 * op:2 * op + 2, sl],
                            start=(op == 0), stop=(op == ho // 2 - 1),
                            perf_mode=DR)
                    # psum is SX * (q|k); rescale to true and add bias
                    nc.vector.scalar_tensor_tensor(
                        dst[:, b, sl], ps, 1.0 / SX,
                        bias_sb[:, 0:1].to_broadcast((P, 512)),
                        mybir.AluOpType.mult, mybir.AluOpType.add)
            for kb in range(nkb):
                ksl = slice(kb * P, (kb + 1) * P)
                ps = ps_qk.tile([P, 512], F32, tag="qk")
                for op in range(ho // 2):
                    nc.tensor.matmul(
                        ps[:, 0:P],
                        lhsT=xT_sb[:, b, 2 * op:2 * op + 2, ksl],
                        rhs=wv_sb[:, 2 * op:2 * op + 2, :],
                        start=(op == 0), stop=(op == ho // 2 - 1),
                        perf_mode=DR)
                # v tiles hold SX * (v + bv); host pre-scales bv by SX
                # (v dims in weight cols 1:65; col 0 is the denominator)
                nc.vector.tensor_tensor(
                    v_e[:, b, kb // 2, kb % 2, 1:65], ps[:, 0:64],
                    bv_b[:, 0:64], mybir.AluOpType.add)
                nc.vector.tensor_tensor(
                    v_o[:, b, kb // 2, kb % 2, 1:65], ps[:, 64:128],
                    bv_b[:, 64:128], mybir.AluOpType.add)

        def attn_stage(b, qcs):
            for qc in qcs:
                qsl = slice(qc * qw, (qc + 1) * qw)
                ctx_e = ps_ctx.tile([P, qw], F32, tag="ctx")
                ctx_o = ps_ctx.tile([P, qw], F32, tag="ctx")
                for kp in range(nkp):
                    pp = ptile.tile([P, 2, 2, qw], FP8, tag="p")
                    for ki in range(2):
                        kb = 2 * kp + ki
                        ksl = slice(kb * P, (kb + 1) * P)
                        sp = ps_s.tile([P, 2, qw], F32, tag="s")
                        nc.tensor.matmul(
                            sp[:, 0, :], lhsT=kT_sb[0:64, b, ksl],
                            rhs=qT_sb[0:64, b, qsl], start=True, stop=True)
                        nc.tensor.matmul(
                            sp[:, 1, :], lhsT=kT_sb[64:128, b, ksl],
                            rhs=qT_sb[64:128, b, qsl], start=True, stop=True)
                        nc.scalar.activation(
                            pp[:, ki], sp, AF.Exp,
                            bias=mask_sb[:, b, kb:kb + 1], scale=SCALE)
                    nc.tensor.matmul(
                        ctx_e[0:65, :], lhsT=v_e[:, b, kp, :, 0:65],
                        rhs=pp[:, :, 0, :], start=(kp == 0), stop=(kp == nkp - 1),
                        perf_mode=DR, skip_group_check=True)
                    nc.tensor.matmul(
                        ctx_o[0:65, :], lhsT=v_o[:, b, kp, :, 0:65],
                        rhs=pp[:, :, 1, :], start=(kp == 0), stop=(kp == nkp - 1),
                        perf_mode=DR, skip_group_check=True)
                # normalize: ctx[d, q] / denom[q]  (denom = PSUM row 0,
                # partition-matched for the fast approx reciprocal).
                # recips first (frees the ctx banks early so the next
                # shard's AV matmuls aren't gated on the full chain);
                # the eb broadcast matmuls overlap the ctx_bf copies.
                last = (b == B - 1 and qc == qc_per_b - 1)
                rbs, ctx_bfs = [], []
                for h, ctx_ps in enumerate((ctx_e, ctx_o)):
                    rb32 = misc.tile([1, qw], F32, tag="rb32")
                    nc.vector.reciprocal_approx_fast(rb32, ctx_ps[0:1, :])
                    # bf16 copy for the broadcast matmul (fp32 matmuls
                    # interleaved in the fp8-DR PE stream are avoided)
                    rb = misc.tile([1, qw], BF16, tag="rb")
                    with nc.allow_low_precision(reason="bf16 1/denom, matches prior bf16 cast"):
                        nc.vector.tensor_copy(out=rb, in_=rb32)
                    rbs.append(rb)
                for h, ctx_ps in enumerate((ctx_e, ctx_o)):
                    # rows 0:65 copied partition-matched; row 0 (denom) is
                    # multiplied but never shipped
                    ctx_bf = misc.tile([65, qw], BF16, tag="cb")
                    if last:
                        # final shard is latency-critical (gates the last
                        # collective); ScalarE is idle after the last exp
                        nc.scalar.copy(out=ctx_bf, in_=ctx_ps[0:65, :])
                    else:
                        nc.vector.tensor_copy(out=ctx_bf, in_=ctx_ps[0:65, :])
                    ctx_bfs.append(ctx_bf)
                for h in range(2):
                    eb = ps_eb.tile([65, qw], F32, tag="eb")
                    nc.tensor.matmul(
                        eb, lhsT=ones_sb[0:1, 0:65], rhs=rbs[h],
                        start=True, stop=True)
                    cn = misc.tile([65, 4, rhb], FP8, tag="cn")
                    nc.vector.tensor_tensor(
                        cn, ctx_bfs[h], eb, mybir.AluOpType.mult)
                    # shard (b, qc) holds half-batch hb = 2b + qc//2, cols for
                    # dest cores 4*(qc%2) .. 4*(qc%2)+3; one DMA covers all 4
                    hb = 2 * b + qc // 2
                    d0 = 4 * (qc % 2)
                    nc.sync.dma_start(
                        a2a_in[hb].rearrange(
                            "(d p) q -> p d q", p=P
                        )[h * 64:(h + 1) * 64, d0:d0 + 4, :],
                        cn[1:65])

        def tail_stage(hb, ctxf, last):
            # Wo + residual + LayerNorm for this half-batch's 128-row tile
            a2a_r = a2a_out[hb].rearrange("(o p) q -> p o q", p=P)
            if last:
                # end path: split across both queues (scalar is free after
                # the exp stream) so the first Wo matmuls start earlier
                nc.sync.dma_start(ctxf[:, 0:ho // 2, :], a2a_r[:, 0:ho // 2, :])
                nc.scalar.dma_start(ctxf[:, ho // 2:, :], a2a_r[:, ho // 2:, :])
            else:
                # scalar queue: the collective completed well before this
                # trigger is reached, so it doesn't block the exp stream --
                # and the 4us small-packet transfer stays off the sync
                # queue, which the shards' cn stores need promptly
                nc.scalar.dma_start(ctxf, a2a_r)
            rt = hb                    # row-tile index in [0, 4)
            res = fin.tile([P, H], F32, tag="res")
            # Wo psum borrows the ps_s pool (both [P, 2, 512]); its ring
            # slot is long-drained by the time a tail runs
            ps2 = ps_s.tile([P, 2, 512], F32, tag="s")
            for nch in range(H // 512):
                nsl = slice(nch * 512, (nch + 1) * 512)
                for op in range(ho // 2):
                    nc.tensor.matmul(
                        ps2[:, nch, :],
                        lhsT=ctxf[:, 2 * op:2 * op + 2, 0:P],
                        rhs=wo_sb[:, 2 * op:2 * op + 2, nsl],
                        start=(op == 0), stop=(op == ho // 2 - 1),
                        perf_mode=DR)
                nc.vector.tensor_tensor(
                    res[:, nsl], ps2[:, nch, :], xres_sb[:, rt, nsl],
                    mybir.AluOpType.add)
            # LayerNorm over H (free axis)
            stats = fin.tile([P, H // 512, 6], F32, tag="st")
            for g in range(H // 512):
                nc.vector.bn_stats(
                    stats[:, g, :], res[:, g * 512:(g + 1) * 512])
            mv = fin.tile([P, 2], F32, tag="mv")
            nc.vector.bn_aggr(out=mv, in_=stats)
            rstd = fin.tile([P, 1], F32, tag="rstd")
            nc.scalar.activation(rstd, mv[:, 1:2], AF.Sqrt, bias=eps_sb)
            nc.vector.reciprocal(rstd, rstd)
            # bf16 epilogue: 2x DVE rate; output quantization (~0.4% rms)
            # stays well under the 2e-2 gate
            u = fin.tile([P, H], BF16, tag="u")
            outt = fin.tile([P, H], BF16, tag="outt")
            if last:
                # latency-critical tail after the last collective: column
                # halves so each half's store overlaps the next half's DVE
                for g in range(2):
                    gs = slice(g * (H // 2), (g + 1) * (H // 2))
                    nc.vector.scalar_tensor_tensor(
                        u[:, gs], res[:, gs], mv[:, 0:1], gamma_b[:, gs],
                        mybir.AluOpType.subtract, mybir.AluOpType.mult)
                    nc.vector.scalar_tensor_tensor(
                        outt[:, gs], u[:, gs], rstd[:, 0:1], beta_b[:, gs],
                        mybir.AluOpType.mult, mybir.AluOpType.add)
                    eng = nc.sync if g == 0 else nc.scalar
                    eng.dma_start(
                        out[rt * P:(rt + 1) * P, gs], outt[:, gs])
            else:
                nc.vector.scalar_tensor_tensor(
                    u, res, mv[:, 0:1], gamma_b,
                    mybir.AluOpType.subtract, mybir.AluOpType.mult)
                nc.vector.scalar_tensor_tensor(
                    outt, u, rstd[:, 0:1], beta_b,
                    mybir.AluOpType.mult, mybir.AluOpType.add)
                nc.sync.dma_start(out[rt * P:(rt + 1) * P, :], outt)

        ctxf = [consts.tile([P, ho, rhb], FP8, name=f"ctxf{hb}", tag=f"ctxf{hb}")
                for hb in range(2 * B)]

        def a2a(hb):
            nc.gpsimd.collective_compute(
                "AllToAll", mybir.AluOpType.bypass,
                replica_groups=[list(range(NCORES))],
                ins=[a2a_in[hb][:].opt()], outs=[a2a_out[hb][:].opt()])

        qkv_stage(0)
        attn_stage(0, (0, 1))
        a2a(0)
        # tail-stage inputs (prefetched well before the first tail runs)
        nc.sync.dma_start(wo_sb, wo[:, :, :])
        nc.sync.dma_start(xres_sb, xres[:, :, :])
        attn_stage(0, (2, 3))
        a2a(1)
        qkv_stage(1)
        attn_stage(1, (0, 1))
        a2a(2)
        # tails interleaved, but each emitted one attention stage AFTER its
        # collective completes: the PE queue is in-order, so a tail's Wo
        # matmuls must only reach the head of the queue once ctxf is
        # already resident (an early tail0 stalled the whole PE ~10us
        # waiting on the collective)
        tail_stage(0, ctxf[0], last=False)
        attn_stage(1, (2,))
        tail_stage(1, ctxf[1], last=False)
        attn_stage(1, (3,))
        # tail2 before a2a(3) in the queues: its DVE work drains during
        # shard 7's exp stream, leaving the post-collective end path to
        # tail3 alone
        tail_stage(2, ctxf[2], last=False)
        a2a(3)
        tail_stage(3, ctxf[3], last=True)


def get_program(s=S):
    key = ("nc", s)
    if key not in _CACHE:
        _CACHE[key] = _build_program(s)
    return _CACHE[key]


def make_in_maps(hidden_states, attention_mask, Wq, bq, Wk, bk, Wv, bv, Wo, bo,
                 ln_gamma, ln_beta):
    """Host-side sharding: build the 8 per-core input maps."""
    f8 = ml_dtypes.float8_e4m3
    hs = np.asarray(hidden_states, dtype=np.float32)
    b_, s_, h_ = hs.shape
    nkb = s_ // P
    rpb = s_ // NCORES

    ho = h_ // P
    # x^T in SBUF layout [p, b, o, s] so DMA packets are 512B-2KB runs
    xT = np.ascontiguousarray(hs.transpose(0, 2, 1) * SX).astype(f8)  # [B,H,S]
    xT_pm = np.ascontiguousarray(
        xT.reshape(b_, ho, P, s_).transpose(2, 0, 1, 3))  # [p, b, o, s]
    Wq = np.asarray(Wq, np.float32)
    Wk = np.asarray(Wk, np.float32)
    Wv = np.asarray(Wv, np.float32)
    wo_f8 = np.ascontiguousarray(
        np.asarray(Wo, np.float32) * 256.0).astype(f8)
    wo_pm = np.ascontiguousarray(
        wo_f8.reshape(ho, P, h_).transpose(1, 0, 2))      # [p, o, n]
    bq = np.asarray(bq, np.float32)
    bk = np.asarray(bk, np.float32)
    bv = np.asarray(bv, np.float32) * SX
    bo = np.asarray(bo, np.float32)
    gamma_bc = np.ascontiguousarray(
        np.broadcast_to(np.asarray(ln_gamma, np.float32)[None, :], (P, H)))
    beta_bc = np.ascontiguousarray(
        np.broadcast_to(np.asarray(ln_beta, np.float32)[None, :], (P, H)))
    mask = np.asarray(attention_mask, np.float32).reshape(b_, s_)
    mask_pm = np.ascontiguousarray(
        mask.reshape(b_, nkb, P).transpose(2, 0, 1))      # [p, b, k]

    in_maps = []
    hpb = s_ // 2   # rows per half-batch (1024)
    rhb = hpb // NCORES
    for c in range(NCORES):
        d0 = c * P
        xres_c = np.concatenate(
            [hs[hb // 2, (hb % 2) * hpb + c * rhb:(hb % 2) * hpb + (c + 1) * rhb, :]
             for hb in range(4)], axis=0)
        xres_pm = ((xres_c + bo[None, :]) * 65536.0).reshape(
            4, P, h_).transpose(1, 0, 2)                  # [p, r, h]
        # wqk [t, p, o, d]; wv [p, o, d]
        wqk_pm = np.stack([Wq[:, d0:d0 + P], Wk[:, d0:d0 + P]]).astype(
            f8).reshape(2, ho, P, P).transpose(0, 2, 1, 3)
        wv_pm = Wv[:, d0:d0 + P].astype(f8).reshape(
            ho, P, P).transpose(1, 0, 2)
        blob8 = np.concatenate([
            xT_pm.ravel(),
            np.ascontiguousarray(wqk_pm).ravel(),
            np.ascontiguousarray(wv_pm).ravel(),
            wo_pm.ravel()])
        blob32 = np.concatenate([
            np.stack([bq[d0:d0 + P], bk[d0:d0 + P]], axis=1).ravel(),
            np.broadcast_to(bv[d0:d0 + P][None, :], (P, P)).ravel(),
            mask_pm.ravel(),
            np.ascontiguousarray(xres_pm).ravel(),
            gamma_bc.ravel(), beta_bc.ravel()]).astype(np.float32)
        in_maps.append({
            "blob": np.concatenate([
                np.ascontiguousarray(blob8),
                np.frombuffer(np.ascontiguousarray(blob32).tobytes(),
                              dtype=f8)]),
        })
    return in_maps


def assemble_output(results, b_=B, s_=S, h_=H):
    hpb = s_ // 2
    rhb = hpb // NCORES
    out = np.empty((b_, s_, h_), np.float32)
    for c in range(NCORES):
        r = np.asarray(results[c]["out"], np.float32)
        for hb in range(4):
            r0 = (hb % 2) * hpb + c * rhb
            out[hb // 2, r0:r0 + rhb, :] = r[hb * rhb:(hb + 1) * rhb]
    return out


def kernel(**inputs):
    nc = get_program(S)
    in_maps = make_in_maps(**inputs)
    res = run_bass_kernel_spmd(nc, in_maps, list(range(NCORES)))
    return assemble_output(res.results)



# revision 34
# speedup vs baseline: 1.0277x; 1.0277x over previous
"""Trainium2 Bass kernel for BaseBertSelfAttention (B=2, S=2048, H=1024, 16 heads).

Sharding (8 NeuronCores):
  - Tensor-parallel on heads: core c owns heads (2c, 2c+1) -> d_local = 128.
  - Each core: QKV projections (column-parallel) for its 2 heads over BOTH
    batches, attention in transposed layout (scores^T: keys on partitions,
    queries on the free axis), softmax denominator via a ones-augmented V
    column (weight column 0, so the denominator lands on PSUM partition 0
    where reciprocal_approx_fast reads it partition-matched -- the custom
    DVE op misreads non-zero base partitions on HW), normalized context
    ctx^T [d_local=128, B*S].
  - FOUR AllToAlls (one per half-batch, 128KB each) redistribute ctx^T from
    head-sharding to row-sharding.  Each one-row-tile Wo/LayerNorm tail is
    emitted one attention stage AFTER its collective completes: the
    per-engine instruction queues are in-order, so a tail's Wo matmuls
    must only reach the head of the PE queue once its ctxf is already
    resident (hardware-measured: an early tail stalls the whole PE ~10us
    behind the collective).  Only the final collective plus one tail
    (~20-30 us, collective-warmth dependent) is exposed at the end.
  - Core c owns rows [128c, 128c+128) of each half-batch (512 rows total).

Hardware-profiled bottleneck structure (NTFF traces, not the simulator --
the sim is ~25% optimistic and mis-ranks engines): the softmax exp stream
(16.8M elements/core, 128 ACTIVATEs of 1024 elem/partition at
0.833ns/elem + 260ns/call) is ~143us of ACT time; the PE runs ~176us
residency (scores already row-packed pairwise via base_partition 0/64 ->
concurrent tiles).  Everything else is latency engineering: startup races
to the first exp at ~20us (framework preamble is 7us; front loads split
across the sync+scalar DMA queues with partition-major host layouts for
>=1KB DMA packets), shard-boundary normalize chains are kept off the
critical path (reciprocal_approx_fast at 0.67us vs 3.35us for the slow
cross-partition reciprocal; copies drain the ctx PSUM banks before the
eb-broadcast/cn multiply chain), and mid-kernel tails run their ctxf loads
on the scalar queue so the sync queue stays clear for shard cn stores.

Precision: fp8 (e4m3) matmul inputs with DoubleRow packing for the QKV
projections, the probs@V matmul and the Wo projection; bf16 Q^T/K^T for
the scores matmul; fp32 PSUM accumulation and softmax denominators; bf16
LayerNorm epilogue and bf16 output (2x DVE rate; host converts back to
f32).  Scale bookkeeping: x^T is pre-scaled by SX=16, Q/K rescale at the
bias-add, V carries SX with a 1/SX denominator column, Wo carries 256 and
the residual 65536 -- the common row scale cancels in LayerNorm.
NaN traps on HW (sim-clean, hardware-broken): fp32 matmuls (FP32HI
two-pass) interleaved into the fp8-DR PE stream, and in-place or
cross-partition reciprocal_approx_fast.

All inputs pack into ONE fp8 blob (f32 section bitcast on device): each
extra PJRT buffer costs ~30-150us/dispatch through the axon tunnel.
Hardware-verified: device exec ~250-258us (NTFF, core 0; was 300.6us),
relative error 2.6e-3 (gate 2e-2).
"""

import numpy as np
import ml_dtypes

import concourse.bass as bass
import concourse.tile as tile
from concourse import bacc, mybir
from concourse.bass_utils import run_bass_kernel_spmd

BF16 = mybir.dt.bfloat16
FP8 = mybir.dt.float8e4
F32 = mybir.dt.float32
AF = mybir.ActivationFunctionType
DR = mybir.MatmulPerfMode.DoubleRow
P = 128

B, S, H = 2, 2048, 1024
NH, HD = 16, 64
NCORES = 8
EPS = 1e-12
SCALE = 1.0 / 8.0   # 1/sqrt(HD)
SX = 16.0           # fp8 x^T pre-scale (host side)
SVC = 1.0 / SX      # denominator column constant: cn = 256 * v_avg

_CACHE: dict = {}


def _build_program(s=S):
    """Build the (identical-across-cores) Bass program."""
    nkb = s // P               # key blocks of 128 (16)
    qc_per_b = NCORES // B     # q chunks per batch (4)
    qw = (B * s) // NCORES     # q-chunk width (512)
    rpb = s // NCORES          # output rows per core per batch (256)
    ho = H // P                # h chunks of 128 (8)

    nc = bacc.Bacc("TRN2", target_bir_lowering=False, debug=False,
                   num_devices=NCORES)
    # All inputs are packed into ONE flat fp8 blob: each extra PJRT input
    # buffer costs ~30-150 us of per-dispatch overhead through the axon
    # tunnel.  The f32 section is stored as raw bytes and bitcast on
    # device (the fp8 section's total size is 4B-aligned).
    n8 = {"xT": B * H * s, "wqk": H * 2 * P, "wv": H * P, "wo": H * H}
    n32 = {"bqk": P * 2, "bv": P * P, "maskT": B * P * nkb,
           "xres": 2 * rpb * H, "gamma": P * H, "beta": P * H}
    N8 = sum(n8.values())
    assert N8 % 4 == 0
    blob = nc.dram_tensor("blob", [N8 + 4 * sum(n32.values())], FP8,
                          kind="ExternalInput")

    def views(sizes, base, scale, cast):
        o, vs = 0, {}
        for k, n in sizes.items():
            ap = blob[base + scale * o:base + scale * (o + n)]
            vs[k] = ap.bitcast(F32) if cast else ap
            o += n
        return vs

    # host-side layouts are partition-major (SBUF layout) so every DMA
    # reads long contiguous runs per partition instead of 128B packets
    v8 = views(n8, 0, 1, cast=False)
    v32 = views(n32, N8, 4, cast=True)
    xT = v8["xT"].rearrange("(p b o ss) -> p b o ss", p=P, b=B, o=H // P)
    wqk = v8["wqk"].rearrange("(t p o d) -> t p o d", t=2, p=P, o=H // P)
    wv = v8["wv"].rearrange("(p o d) -> p o d", p=P, o=H // P)
    wo = v8["wo"].rearrange("(p o n) -> p o n", p=P, o=H // P)
    bqk = v32["bqk"].rearrange("(p t) -> p t", p=P)
    bv = v32["bv"].rearrange("(p d) -> p d", p=P)
    maskT = v32["maskT"].rearrange("(p b k) -> p b k", p=P, b=B)
    xres = v32["xres"].rearrange("(p r hh) -> p r hh", p=P, hh=H)
    gamma = v32["gamma"].rearrange("(p hh) -> p hh", p=P)
    beta = v32["beta"].rearrange("(p hh) -> p hh", p=P)
    out = nc.dram_tensor("out", [2 * rpb, H], BF16, kind="ExternalOutput")

    with tile.TileContext(nc) as tc:
        _kernel_body(
            tc, s, nkb, qw, qc_per_b, rpb, ho,
            xT, wqk, wv, wo, bqk, bv, maskT, xres, gamma, beta, out,
        )
    nc.compile()
    return nc


def _kernel_body(tc, s, nkb, qw, qc_per_b, rpb, ho,
                 xT, wqk, wv, wo, bqk, bv, maskT, xres, gamma, beta, out):
    nc = tc.nc
    VPAD = 80  # padded free width of the ones-augmented V tiles (65 used)
    nkp = nkb // 2  # key-block pairs (8) for DoubleRow AV

    import contextlib
    stack = contextlib.ExitStack()
    with stack:
        consts = stack.enter_context(tc.tile_pool(name="consts", bufs=1))
        dram = stack.enter_context(tc.tile_pool(name="dram", bufs=1, space="DRAM"))

        # ---------------- constant / input loads ----------------
        # Startup is latency-critical (the softmax ScalarEngine stream is the
        # kernel bottleneck, so its first exp should start ASAP).  Split the
        # front loads across BOTH DMA queues: sync (SP) takes wqk + the even
        # x^T chunks, the scalar-engine queue (idle before the first exp)
        # takes bqk/mask + the odd x^T chunks + V-path constants.
        wqk_sb = consts.tile([P, ho, 2, P], FP8)
        # two DMAs into one tile: the Q half lands first so the very first
        # projection matmul isn't gated on the K half
        nc.sync.dma_start(wqk_sb[:, :, 0, :], wqk[0])
        nc.sync.dma_start(wqk_sb[:, :, 1, :], wqk[1])
        wq_sb = wqk_sb[:, :, 0, :]
        wk_sb = wqk_sb[:, :, 1, :]

        bqk_sb = consts.tile([P, 2], F32)
        nc.scalar.dma_start(bqk_sb, bqk[:, :])
        bq_sb = bqk_sb[:, 0:1]
        bk_sb = bqk_sb[:, 1:2]
        mask_sb = consts.tile([P, B, nkb], F32)
        nc.scalar.dma_start(mask_sb, maskT[:, :, :])

        xT_sb = consts.tile([P, B, ho, s], FP8)
        xT_r = xT
        # x^T (fp8, host-scaled by SX): first s-chunk of batch 0 across all o
        for o in range(0, ho, 2):
            nc.sync.dma_start(xT_sb[:, 0, o, 0:512], xT_r[:, 0, o, 0:512])
            nc.scalar.dma_start(
                xT_sb[:, 0, o + 1, 0:512], xT_r[:, 0, o + 1, 0:512])

        wv_sb = consts.tile([P, ho, P], FP8)
        nc.scalar.dma_start(wv_sb, wv[:, :, :])
        bv_b = consts.tile([P, P], F32)
        nc.scalar.dma_start(bv_b, bv[:, :])

        # rest of batch-0 x^T, then batch 1 (sync queue; scalar queue must
        # stay clear once the softmax exp stream begins)
        for o in range(ho):
            nc.sync.dma_start(xT_sb[:, 0, o, 512:s], xT_r[:, 0, o, 512:s])
        for o in range(ho):
            nc.sync.dma_start(xT_sb[:, 1, o, :], xT_r[:, 1, o, :])

        wo_sb = consts.tile([P, ho, H], FP8)
        ones_sb = consts.tile([1, 65], BF16)
        nc.vector.memset(ones_sb, 1.0)
        eps_sb = consts.tile([P, 1], F32)
        nc.vector.memset(eps_sb, EPS)

        # gamma/beta on the gpsimd DMA queue (idle at startup; only needed
        # by the tails), keeping the sync/scalar queues for the race to the
        # first exp
        gamma_b = consts.tile([P, H], F32)
        nc.sync.dma_start(gamma_b, gamma[:, :])
        beta_b = consts.tile([P, H], F32)
        nc.sync.dma_start(beta_b, beta[:, :])

        xres_sb = consts.tile([P, 2 * rpb // P, H], F32)

        # attention intermediates
        qT_sb = consts.tile([P, B, s], BF16)   # Q^T [d_local, b, s] true scale
        kT_sb = consts.tile([P, B, s], BF16)   # K^T [d_local, b, s] true scale
        # ones-augmented V (natural layout), per head:
        #   [p(s-inner), b, kb-pair, 2, VPAD] fp8, value scale SX
        v_e = consts.tile([P, B, nkp, 2, VPAD], FP8)
        v_o = consts.tile([P, B, nkp, 2, VPAD], FP8)
        # only the denominator column needs initialization (cols 1:65 are
        # overwritten by the V bias-add; cols 65: are never read):
        # cn = ctx*recip lands at 256*v_avg in fp8 range.  The denominator
        # sits in WEIGHT COLUMN 0 so it lands on PSUM PARTITION 0, where
        # reciprocal_approx_fast reads it on the fast partition-matched path
        # (the custom DVE op misreads non-zero base partitions).
        nc.vector.memset(v_e[:, :, :, :, 0:1], SVC)
        nc.vector.memset(v_o[:, :, :, :, 0:1], SVC)

        # A2A bounce buffers (DRAM, local), one pair per HALF-batch: four
        # small collectives let the first three (and their tails) hide under
        # the softmax plateau; only the last 128KB exchange is exposed
        rhb = rpb // 2   # rows per core per half-batch (128)
        a2a_in = [dram.tile([NCORES * P, rhb], FP8, name=f"a2a_in{hb}")
                  for hb in range(2 * B)]
        a2a_out = [dram.tile([NCORES * P, rhb], FP8, name=f"a2a_out{hb}")
                   for hb in range(2 * B)]

        # PSUM: qk pool 1 bank (QKV proj), s pool 2x2 banks (shared with the
        # tails' Wo matmuls), ctx pool 2 banks, eb pool 1 bank = 8 banks.
        # Keeping eb out of ps_ctx lets the next shard's AV accumulation
        # start as soon as the ctx tiles are drained (copy+recip), instead
        # of waiting for the full normalize chain.
        ps_qk = stack.enter_context(tc.tile_pool(name="ps_qk", bufs=1, space="PSUM"))
        ps_s = stack.enter_context(tc.tile_pool(name="ps_s", bufs=2, space="PSUM"))
        ps_ctx = stack.enter_context(tc.tile_pool(name="ps_ctx", bufs=2, space="PSUM"))
        ps_eb = stack.enter_context(tc.tile_pool(name="ps_eb", bufs=1, space="PSUM"))
        ptile = stack.enter_context(tc.tile_pool(name="ptile", bufs=6))
        misc = stack.enter_context(tc.tile_pool(name="misc", bufs=3))
        fin = stack.enter_context(tc.tile_pool(name="fin", bufs=2))

        def qkv_stage(b):
            # Q/K chunk projections first (attention's scores need them
            # immediately); V blocks after (first consumed only after the
            # first softmax exp)
            for sc in range(s // 512):
                sl = slice(sc * 512, (sc + 1) * 512)
                for w_sb, bias_sb, dst in (
                    (wq_sb, bq_sb, qT_sb),
                    (wk_sb, bk_sb, kT_sb),
                ):
                    ps = ps_qk.tile([P, 512], F32, tag="qk")
                    for op in range(ho // 2):
                        nc.tensor.matmul(
                            ps, lhsT=w_sb[:, 2 * op:2 * op + 2, :].opt(),
                            rhs=xT_sb[:, b, 2 * op:2 * op + 2, sl],
                            start=(op == 0), stop=(op == ho // 2 - 1),
                            perf_mode=DR)
                    # psum is SX * (q|k); rescale to true and add bias
                    nc.vector.scalar_tensor_tensor(
                        dst[:, b, sl], ps, 1.0 / SX,
                        bias_sb[:, 0:1].to_broadcast((P, 512)),
                        mybir.AluOpType.mult, mybir.AluOpType.add)
            for kb in range(nkb):
                ksl = slice(kb * P, (kb + 1) * P)
                ps = ps_qk.tile([P, 512], F32, tag="qk")
                for op in range(ho // 2):
                    nc.tensor.matmul(
                        ps[:, 0:P],
                        lhsT=xT_sb[:, b, 2 * op:2 * op + 2, ksl],
                        rhs=wv_sb[:, 2 * op:2 * op + 2, :],
                        start=(op == 0), stop=(op == ho // 2 - 1),
                        perf_mode=DR)
                # v tiles hold SX * (v + bv); host pre-scales bv by SX
                # (v dims in weight cols 1:65; col 0 is the denominator)
                nc.vector.tensor_tensor(
                    v_e[:, b, kb // 2, kb % 2, 1:65], ps[:, 0:64],
                    bv_b[:, 0:64], mybir.AluOpType.add)
                nc.vector.tensor_tensor(
                    v_o[:, b, kb // 2, kb % 2, 1:65], ps[:, 64:128],
                    bv_b[:, 64:128], mybir.AluOpType.add)

        def attn_stage(b, qcs):
            for qc in qcs:
                qsl = slice(qc * qw, (qc + 1) * qw)
                ctx_e = ps_ctx.tile([P, qw], F32, tag="ctx")
                ctx_o = ps_ctx.tile([P, qw], F32, tag="ctx")
                for kp in range(nkp):
                    pp = ptile.tile([P, 2, 2, qw], FP8, tag="p")
                    for ki in range(2):
                        kb = 2 * kp + ki
                        ksl = slice(kb * P, (kb + 1) * P)
                        sp = ps_s.tile([P, 2, qw], F32, tag="s")
                        nc.tensor.matmul(
                            sp[:, 0, :], lhsT=kT_sb[0:64, b, ksl],
                            rhs=qT_sb[0:64, b, qsl], start=True, stop=True)
                        nc.tensor.matmul(
                            sp[:, 1, :], lhsT=kT_sb[64:128, b, ksl],
                            rhs=qT_sb[64:128, b, qsl], start=True, stop=True)
                        nc.scalar.activation(
                            pp[:, ki], sp, AF.Exp,
                            bias=mask_sb[:, b, kb:kb + 1], scale=SCALE)
                    nc.tensor.matmul(
                        ctx_e[0:65, :], lhsT=v_e[:, b, kp, :, 0:65],
                        rhs=pp[:, :, 0, :], start=(kp == 0), stop=(kp == nkp - 1),
                        perf_mode=DR, skip_group_check=True)
                    nc.tensor.matmul(
                        ctx_o[0:65, :], lhsT=v_o[:, b, kp, :, 0:65],
                        rhs=pp[:, :, 1, :], start=(kp == 0), stop=(kp == nkp - 1),
                        perf_mode=DR, skip_group_check=True)
                # normalize: ctx[d, q] / denom[q]  (denom = PSUM row 0,
                # partition-matched for the fast approx reciprocal).
                # recips first (frees the ctx banks early so the next
                # shard's AV matmuls aren't gated on the full chain);
                # the eb broadcast matmuls overlap the ctx_bf copies.
                last = (b == B - 1 and qc == qc_per_b - 1)
                rbs, ctx_bfs = [], []
                for h, ctx_ps in enumerate((ctx_e, ctx_o)):
                    rb32 = misc.tile([1, qw], F32, tag="rb32")
                    nc.vector.reciprocal_approx_fast(rb32, ctx_ps[0:1, :])
                    # bf16 copy for the broadcast matmul (fp32 matmuls
                    # interleaved in the fp8-DR PE stream are avoided)
                    rb = misc.tile([1, qw], BF16, tag="rb")
                    with nc.allow_low_precision(reason="bf16 1/denom, matches prior bf16 cast"):
                        nc.vector.tensor_copy(out=rb, in_=rb32)
                    rbs.append(rb)
                for h, ctx_ps in enumerate((ctx_e, ctx_o)):
                    # rows 0:65 copied partition-matched; row 0 (denom) is
                    # multiplied but never shipped
                    ctx_bf = misc.tile([65, qw], BF16, tag="cb")
                    if last:
                        # final shard is latency-critical (gates the last
                        # collective); ScalarE is idle after the last exp
                        nc.scalar.copy(out=ctx_bf, in_=ctx_ps[0:65, :])
                    else:
                        nc.vector.tensor_copy(out=ctx_bf, in_=ctx_ps[0:65, :])
                    ctx_bfs.append(ctx_bf)
                for h in range(2):
                    eb = ps_eb.tile([65, qw], F32, tag="eb")
                    nc.tensor.matmul(
                        eb, lhsT=ones_sb[0:1, 0:65], rhs=rbs[h],
                        start=True, stop=True)
                    cn = misc.tile([65, 4, rhb], FP8, tag="cn")
                    nc.vector.tensor_tensor(
                        cn, ctx_bfs[h], eb, mybir.AluOpType.mult)
                    # shard (b, qc) holds half-batch hb = 2b + qc//2, cols for
                    # dest cores 4*(qc%2) .. 4*(qc%2)+3; one DMA covers all 4
                    hb = 2 * b + qc // 2
                    d0 = 4 * (qc % 2)
                    nc.sync.dma_start(
                        a2a_in[hb].rearrange(
                            "(d p) q -> p d q", p=P
                        )[h * 64:(h + 1) * 64, d0:d0 + 4, :],
                        cn[1:65])

        def tail_stage(hb, ctxf, last):
            # Wo + residual + LayerNorm for this half-batch's 128-row tile
            a2a_r = a2a_out[hb].rearrange("(o p) q -> p o q", p=P)
            if last:
                # end path: split across both queues (scalar is free after
                # the exp stream) so the first Wo matmuls start earlier
                nc.sync.dma_start(ctxf[:, 0:ho // 2, :], a2a_r[:, 0:ho // 2, :])
                nc.scalar.dma_start(ctxf[:, ho // 2:, :], a2a_r[:, ho // 2:, :])
            else:
                # scalar queue: the collective completed well before this
                # trigger is reached, so it doesn't block the exp stream --
                # and the 4us small-packet transfer stays off the sync
                # queue, which the shards' cn stores need promptly
                nc.scalar.dma_start(ctxf, a2a_r)
            rt = hb                    # row-tile index in [0, 4)
            res = fin.tile([P, H], F32, tag="res")
            # Wo psum borrows the ps_s pool (both [P, 2, 512]); its ring
            # slot is long-drained by the time a tail runs
            ps2 = ps_s.tile([P, 2, 512], F32, tag="s")
            for nch in range(H // 512):
                nsl = slice(nch * 512, (nch + 1) * 512)
                for op in range(ho // 2):
                    nc.tensor.matmul(
                        ps2[:, nch, :],
                        lhsT=ctxf[:, 2 * op:2 * op + 2, 0:P],
                        rhs=wo_sb[:, 2 * op:2 * op + 2, nsl],
                        start=(op == 0), stop=(op == ho // 2 - 1),
                        perf_mode=DR)
                nc.vector.tensor_tensor(
                    res[:, nsl], ps2[:, nch, :], xres_sb[:, rt, nsl],
                    mybir.AluOpType.add)
            # LayerNorm over H (free axis)
            stats = fin.tile([P, H // 512, 6], F32, tag="st")
            for g in range(H // 512):
                nc.vector.bn_stats(
                    stats[:, g, :], res[:, g * 512:(g + 1) * 512])
            mv = fin.tile([P, 2], F32, tag="mv")
            nc.vector.bn_aggr(out=mv, in_=stats)
            rstd = fin.tile([P, 1], F32, tag="rstd")
            nc.scalar.activation(rstd, mv[:, 1:2], AF.Sqrt, bias=eps_sb)
            nc.vector.reciprocal(rstd, rstd)
            # bf16 epilogue: 2x DVE rate; output quantization (~0.4% rms)
            # stays well under the 2e-2 gate
            u = fin.tile([P, H], BF16, tag="u")
            outt = fin.tile([P, H], BF16, tag="outt")
            if last:
                # latency-critical tail after the last collective: column
                # halves so each half's store overlaps the next half's DVE
                for g in range(2):
                    gs = slice(g * (H // 2), (g + 1) * (H // 2))
                    nc.vector.scalar_tensor_tensor(
                        u[:, gs], res[:, gs], mv[:, 0:1], gamma_b[:, gs],
                        mybir.AluOpType.subtract, mybir.AluOpType.mult)
                    nc.vector.scalar_tensor_tensor(
                        outt[:, gs], u[:, gs], rstd[:, 0:1], beta_b[:, gs],
                        mybir.AluOpType.mult, mybir.AluOpType.add)
                    eng = nc.sync if g == 0 else nc.scalar
                    eng.dma_start(
                        out[rt * P:(rt + 1) * P, gs], outt[:, gs])
            else:
                nc.vector.scalar_tensor_tensor(
                    u, res, mv[:, 0:1], gamma_b,
                    mybir.AluOpType.subtract, mybir.AluOpType.mult)
                nc.vector.scalar_tensor_tensor(
                    outt, u, rstd[:, 0:1], beta_b,
                    mybir.AluOpType.mult, mybir.AluOpType.add)
                nc.sync.dma_start(out[rt * P:(rt + 1) * P, :], outt)

        ctxf = [consts.tile([P, ho, rhb], FP8, name=f"ctxf{hb}", tag=f"ctxf{hb}")
                for hb in range(2 * B)]

        def a2a(hb):
            nc.gpsimd.collective_compute(
                "AllToAll", mybir.AluOpType.bypass,
                replica_groups=[list(range(NCORES))],
                ins=[a2a_in[hb][:].opt()], outs=[a2a_out[hb][:].opt()])

        qkv_stage(0)
        attn_stage(0, (0, 1))
        a2a(0)
        # tail-stage inputs (prefetched well before the first tail runs)
        nc.sync.dma_start(wo_sb, wo[:, :, :])
        nc.sync.dma_start(xres_sb, xres[:, :, :])
        attn_stage(0, (2, 3))
        a2a(1)
        qkv_stage(1)
        attn_stage(1, (0, 1))
        a2a(2)
        # tails interleaved, but each emitted one attention stage AFTER its
        # collective completes: the PE queue is in-order, so a tail's Wo
        # matmuls must only reach the head of the queue once ctxf is
        # already resident (an early tail0 stalled the whole PE ~10us
        # waiting on the collective)
        tail_stage(0, ctxf[0], last=False)
        attn_stage(1, (2,))
        tail_stage(1, ctxf[1], last=False)
        attn_stage(1, (3,))
        # tail2 before a2a(3) in the queues: its DVE work drains during
        # shard 7's exp stream, leaving the post-collective end path to
        # tail3 alone
        tail_stage(2, ctxf[2], last=False)
        a2a(3)
        tail_stage(3, ctxf[3], last=True)


def get_program(s=S):
    key = ("nc", s)
    if key not in _CACHE:
        _CACHE[key] = _build_program(s)
    return _CACHE[key]


def make_in_maps(hidden_states, attention_mask, Wq, bq, Wk, bk, Wv, bv, Wo, bo,
                 ln_gamma, ln_beta):
    """Host-side sharding: build the 8 per-core input maps."""
    f8 = ml_dtypes.float8_e4m3
    hs = np.asarray(hidden_states, dtype=np.float32)
    b_, s_, h_ = hs.shape
    nkb = s_ // P
    rpb = s_ // NCORES

    ho = h_ // P
    # x^T in SBUF layout [p, b, o, s] so DMA packets are 512B-2KB runs
    xT = np.ascontiguousarray(hs.transpose(0, 2, 1) * SX).astype(f8)  # [B,H,S]
    xT_pm = np.ascontiguousarray(
        xT.reshape(b_, ho, P, s_).transpose(2, 0, 1, 3))  # [p, b, o, s]
    Wq = np.asarray(Wq, np.float32)
    Wk = np.asarray(Wk, np.float32)
    Wv = np.asarray(Wv, np.float32)
    wo_f8 = np.ascontiguousarray(
        np.asarray(Wo, np.float32) * 256.0).astype(f8)
    wo_pm = np.ascontiguousarray(
        wo_f8.reshape(ho, P, h_).transpose(1, 0, 2))      # [p, o, n]
    bq = np.asarray(bq, np.float32)
    bk = np.asarray(bk, np.float32)
    bv = np.asarray(bv, np.float32) * SX
    bo = np.asarray(bo, np.float32)
    gamma_bc = np.ascontiguousarray(
        np.broadcast_to(np.asarray(ln_gamma, np.float32)[None, :], (P, H)))
    beta_bc = np.ascontiguousarray(
        np.broadcast_to(np.asarray(ln_beta, np.float32)[None, :], (P, H)))
    mask = np.asarray(attention_mask, np.float32).reshape(b_, s_)
    mask_pm = np.ascontiguousarray(
        mask.reshape(b_, nkb, P).transpose(2, 0, 1))      # [p, b, k]

    in_maps = []
    hpb = s_ // 2   # rows per half-batch (1024)
    rhb = hpb // NCORES
    for c in range(NCORES):
        d0 = c * P
        xres_c = np.concatenate(
            [hs[hb // 2, (hb % 2) * hpb + c * rhb:(hb % 2) * hpb + (c + 1) * rhb, :]
             for hb in range(4)], axis=0)
        xres_pm = ((xres_c + bo[None, :]) * 65536.0).reshape(
            4, P, h_).transpose(1, 0, 2)                  # [p, r, h]
        # wqk [t, p, o, d]; wv [p, o, d]
        wqk_pm = np.stack([Wq[:, d0:d0 + P], Wk[:, d0:d0 + P]]).astype(
            f8).reshape(2, ho, P, P).transpose(0, 2, 1, 3)
        wv_pm = Wv[:, d0:d0 + P].astype(f8).reshape(
            ho, P, P).transpose(1, 0, 2)
        blob8 = np.concatenate([
            xT_pm.ravel(),
            np.ascontiguousarray(wqk_pm).ravel(),
            np.ascontiguousarray(wv_pm).ravel(),
            wo_pm.ravel()])
        blob32 = np.concatenate([
            np.stack([bq[d0:d0 + P], bk[d0:d0 + P]], axis=1).ravel(),
            np.broadcast_to(bv[d0:d0 + P][None, :], (P, P)).ravel(),
            mask_pm.ravel(),
            np.ascontiguousarray(xres_pm).ravel(),
            gamma_bc.ravel(), beta_bc.ravel()]).astype(np.float32)
        in_maps.append({
            "blob": np.concatenate([
                np.ascontiguousarray(blob8),
                np.frombuffer(np.ascontiguousarray(blob32).tobytes(),
                              dtype=f8)]),
        })
    return in_maps


def assemble_output(results, b_=B, s_=S, h_=H):
    hpb = s_ // 2
    rhb = hpb // NCORES
    out = np.empty((b_, s_, h_), np.float32)
    for c in range(NCORES):
        r = np.asarray(results[c]["out"], np.float32)
        for hb in range(4):
            r0 = (hb % 2) * hpb + c * rhb
            out[hb // 2, r0:r0 + rhb, :] = r[hb * rhb:(hb + 1) * rhb]
    return out


def kernel(**inputs):
    nc = get_program(S)
    in_maps = make_in_maps(**inputs)
    res = run_bass_kernel_spmd(nc, in_maps, list(range(NCORES)))
    return assemble_output(res.results)

